# revision 42
# baseline (speedup 1.0000x reference)
"""GraphSAGE-style 2-layer GNN minibatch forward on 8 trn2 NeuronCores.

Data-parallel over the 1024 target nodes: each core handles 128 targets.

The host pre-expands the 2-level node tree into a per-core, per-group
feature stream laid out TRANSPOSED ([feature, slot, chunk, token]) so
the device does no gather at all: each 128-token group is one linear
~0.9 MB dma_start on the SP HWDGE ring (full HBM bandwidth; consts ride
the ACT ring so they never block the stream). Self rows travel in bf16,
neighbor rows in fp8-e4m3 (they only enter through a mean of 25, which
washes out the quantization noise; measured end-to-end rel err ~4e-3).

Neighbor aggregation runs on the PE as identity-weight DoubleRow
matmuls (two fp8 slots summed per streamed column) accumulating in
PSUM; the [feat, token] result is exactly the lhsT layout the MLP
matmuls need, so there are no on-device transposes and no
(1x-mode-capped) DVE tensor_reduce on the hot path. The mean /S is
folded into the weight matrices on the host. The MLP runs one group
behind the aggregation (software pipeline) so the PSUM->SBUF copy never
stalls the PE, layer-1 aggregation is computed directly in transposed
form (lhsT = h1 tiles) interleaved into the group loop, and dummy
warm-up matmuls hold the PE's HAM activity monitor at the 2.4 GHz
p-state through the cold start and the tail.

All shapes hardcoded; self-contained (only needs the concourse runtime
that ships with the container).
"""

import numpy as np

N_CORES = 8
N_NODES = 100000
D = 256          # feature dim
P = 128          # partitions / tokens per group
B = 1024         # total targets
S0 = 25          # layer-0 fanout
S1 = 10          # layer-1 fanout
NG = 11          # groups of 128 tokens per core at layer 1 (1408 = 11*128)
SBYTES = 2 * P * 2 + S0 * 2 * P   # 6912 stream bytes/partition: self bf16 + neigh fp8
NC0 = 128        # first const tile columns (bf16): DoubleRow identity only
NCA = 1408       # critical const tile columns (bf16): w0, ones, b0
NCB = 2688       # deferred const tile columns (bf16): w1, a1, ident, b1

_CACHE = {}


def _build_program():
    import concourse.bacc as bacc
    import concourse.mybir as mybir
    import concourse.tile as tile

    F32 = mybir.dt.float32
    BF16 = mybir.dt.bfloat16
    FP8 = mybir.dt.float8e4
    I8 = mybir.dt.int8
    AF = mybir.ActivationFunctionType
    PM = mybir.MatmulPerfMode
    ALU = mybir.AluOpType

    nc = bacc.Bacc("TRN2", target_bir_lowering=False, debug=False)

    st_d = nc.dram_tensor("st", [NG, P, SBYTES], I8, kind="ExternalInput")
    cst0_d = nc.dram_tensor("cst0", [P, NC0], BF16, kind="ExternalInput")
    csta_d = nc.dram_tensor("csta", [P, NCA], BF16, kind="ExternalInput")
    cstb_d = nc.dram_tensor("cstb", [P, NCB], BF16, kind="ExternalInput")
    out_d = nc.dram_tensor("out", [P, D], F32, kind="ExternalOutput")

    with tile.TileContext(nc) as tc:
        with (
            tc.tile_pool(name="consts", bufs=1) as consts,
            tc.tile_pool(name="gatp", bufs=6) as gatp,
            tc.tile_pool(name="aggp", bufs=3) as aggp,
            tc.tile_pool(name="xtp", bufs=1) as xtp,
            tc.tile_pool(name="epip", bufs=4) as epip,
            tc.tile_pool(name="aggps", bufs=3, space="PSUM") as aggps,
            tc.tile_pool(name="mmp", bufs=3, space="PSUM") as mmp,
            tc.tile_pool(name="l1ps", bufs=1, space="PSUM") as l1ps,
        ):
            # consts on the ACT HWDGE ring: the tiny DoubleRow identity first
            # (its sem gates group 0's aggregation matmuls), then the rest;
            # the feature stream rides the SP ring alone so its first bytes
            # move as soon as the preamble finishes
            cst0 = consts.tile([P, NC0], BF16, tag="cst0")
            nc.scalar.dma_start(out=cst0[:], in_=cst0_d[:])
            csta = consts.tile([P, NCA], BF16, tag="csta")
            nc.scalar.dma_start(out=csta[:], in_=csta_d[:])

            pend = {}

            def load_group(g):
                t = gatp.tile([P, SBYTES], I8, tag="gat")
                nc.sync.dma_start(out=t[:], in_=st_d[g])
                return t

            for g in range(4):
                pend[g] = load_group(g)

            cstb = consts.tile([P, NCB], BF16, tag="cstb")
            nc.scalar.dma_start(out=cstb[:], in_=cstb_d[:])

            id2 = cst0[:, 0:NC0].bitcast(FP8)              # [P, 2*P] fp8
            id2_dr = id2.rearrange("p (j m) -> p j m", j=2)
            id1_8 = id2[:, 0:P]                            # [P, P] fp8 identity
            w0_sb = [csta[:, c * D:(c + 1) * D] for c in range(4)]
            ones1 = csta[0:1, 1024:1152]                   # [1, P] bf16
            b0_sb = csta[0:1, 1152:1408]                   # [1, D] bf16
            w1_sb = [cstb[:, c * D:(c + 1) * D] for c in range(4)]
            a1_sb = [cstb[:, 1024 + j * P:1024 + (j + 1) * P] for j in range(S1)]
            ident = cstb[:, 2304:2432]                     # [P, P] bf16
            b1_sb = cstb[0:1, 2432:2688]                   # [1, D] bf16

            # scratch + eps on the otherwise-idle GpSimd engine so the DVE
            # sem lane carries only loop ops
            scr = consts.tile([P, D], BF16, tag="scr")
            nc.gpsimd.memset(scr[:], 0.0)
            eps = consts.tile([P, 1], F32, tag="eps")
            nc.gpsimd.memset(eps[:], 1e-30)

            def warm(n):
                # dummy matmuls that keep the PE activity monitor busy so
                # the clock gate stays at (or ramps to) 2.4 GHz; they borrow
                # a rotating ph PSUM buffer (never read)
                junk = mmp.tile([P, D], F32, tag="ph", name="junk")
                for _ in range(n):
                    nc.tensor.matmul(
                        out=junk[:], lhsT=scr[:, 0:P], rhs=scr[:],
                        start=True, stop=True,
                    )

            h1_sb = [
                consts.tile([P, D], BF16, tag=f"h1_{g}", name=f"h1_{g}")
                for g in range(NG)
            ]
            out_sb = consts.tile([P, D], F32, tag="out_sb")

            def epilogue(ph, out_t):
                # out_t = l2norm(relu(ph)) per token (partition); relu runs
                # on the DVE so the ph PSUM bank frees without waiting on the
                # ACT queue (its reuse gates the next group's matmuls)
                h1r = epip.tile([P, D], BF16, tag="h1r")
                nc.vector.tensor_scalar_max(h1r[:], ph[:], 0.0)
                trash = epip.tile([P, D], BF16, tag="trash")
                n2 = epip.tile([P, 1], F32, tag="n2")
                nc.scalar.activation(
                    out=trash[:], in_=h1r[:], func=AF.Square, accum_out=n2[:]
                )
                nrm = epip.tile([P, 1], F32, tag="nrm")
                nc.scalar.activation(out=nrm[:], in_=n2[:], func=AF.Sqrt, bias=eps[:])
                rinv = epip.tile([P, 1], F32, tag="rinv")
                nc.vector.reciprocal(out=rinv[:], in_=nrm[:])
                # scale by 1/norm on DVE (per-partition scalar) to keep the
                # near-saturated ACT engine off the critical path
                nc.vector.tensor_scalar_mul(out_t[:], h1r[:], rinv[:])

            def mlp(ph, xts, w_sb, b_sb):
                nc.tensor.matmul(
                    out=ph[:], lhsT=ones1, rhs=b_sb, start=True, stop=False
                )
                for i, x in enumerate(xts):
                    nc.tensor.matmul(
                        out=ph[:], lhsT=x, rhs=w_sb[i], start=False, stop=(i == 3)
                    )

            def agg_mms(nb, pagg):
                # neighbor sum on PE: 2 fp8 slots per DoubleRow matmul
                for k in range(S0 // 2):
                    nc.tensor.matmul(
                        out=pagg[:], lhsT=id2_dr,
                        rhs=nb[:, k * 2 * D:(k + 1) * 2 * D].rearrange(
                            "p (j n) -> p j n", j=2
                        ),
                        start=(k == 0), stop=False, perf_mode=PM.DoubleRow,
                    )
                nc.tensor.matmul(
                    out=pagg[:], lhsT=id1_8, rhs=nb[:, (S0 - 1) * D:S0 * D],
                    start=False, stop=True,
                )

            # layer-1 transposed aggregation accumulators + lhsT tiles
            # (one PSUM tile per feature chunk: matmul start=True clears
            # has_written bank-wide, so the chunks must not share a bank)
            agg1t = [
                l1ps.tile([P, P], F32, tag=f"agg1t{c}", name=f"agg1t{c}")
                for c in range(2)
            ]
            xts1 = [
                xtp.tile([P, P], BF16, tag=f"xt{i}", name=f"xt{i}")
                for i in range(4)
            ]

            def a1t_mms(j, stop):
                # layer-1 aggregation, transposed: agg1t[f, tgt] accumulates
                # h1[1+j].T @ a1[j] chunk-wise
                for c in range(2):
                    nc.tensor.matmul(
                        out=agg1t[c][:],
                        lhsT=h1_sb[1 + j][:, c * P:(c + 1) * P],
                        rhs=a1_sb[j],
                        start=(j == 0), stop=stop,
                    )

            # hold the PE busy through the cold start so HAM promotes the
            # clock right as group 0's data lands
            warm(26)

            # ---- layer 0: 11 groups, MLP pipelined one group behind; the
            # MLP runs FIRST in each iteration so its epilogue (which frees
            # the ph PSUM bank) overlaps the long aggregation block ----
            prev = None
            for g in range(NG):
                gat = pend.pop(g)
                if g + 4 < NG:
                    pend[g + 4] = load_group(g + 4)
                self_bf = gat[:, 0:2 * P * 2].bitcast(BF16)      # [P, 2*P]
                nb = gat[:, 2 * P * 2:SBYTES].bitcast(FP8)       # [P, S0*2*P]
                if prev is not None:
                    pself, paggs, pg = prev
                    ph = mmp.tile([P, D], F32, tag="ph")
                    mlp(
                        ph,
                        [pself[:, 0:P], pself[:, P:2 * P],
                         paggs[:, 0:P], paggs[:, P:2 * P]],
                        w0_sb, b0_sb,
                    )
                    epilogue(ph, h1_sb[pg])
                pagg = aggps.tile([P, D], F32, tag="pagg")
                agg_mms(nb, pagg)
                aggs = aggp.tile([P, D], BF16, tag="aggs")
                nc.vector.tensor_copy(out=aggs[:], in_=pagg[:])
                if prev is not None:
                    if g >= 4:
                        a1t_mms(g - 4, stop=False)
                    if g == 3:
                        # transpose layer-1 self (h1[0]) into lhsT layout via
                        # identity-rhs matmuls, borrowing the agg1t PSUM
                        # banks (their accumulation only opens at g == 4)
                        for i in range(2):
                            nc.tensor.matmul(
                                out=agg1t[i][:],
                                lhsT=h1_sb[0][:, i * P:(i + 1) * P],
                                rhs=ident, start=True, stop=True,
                            )
                            nc.vector.tensor_copy(
                                out=xts1[i][:], in_=agg1t[i][:]
                            )
                prev = (self_bf, aggs, g)

            # ---- drain the pipeline: group 10's MLP ----
            warm(2)
            pself, paggs, pg = prev
            ph = mmp.tile([P, D], F32, tag="ph")
            mlp(
                ph,
                [pself[:, 0:P], pself[:, P:2 * P],
                 paggs[:, 0:P], paggs[:, P:2 * P]],
                w0_sb, b0_sb,
            )
            epilogue(ph, h1_sb[pg])
            a1t_mms(S1 - 3, stop=False)      # h1[8] is long done
            a1t_mms(S1 - 2, stop=False)      # h1[9] is long done
            warm(10)                         # PE stays hot while epilogue runs
            a1t_mms(S1 - 1, stop=True)       # needs h1[10]

            # ---- layer 1 tail ----
            for c in range(2):
                nc.vector.tensor_copy(out=xts1[2 + c][:], in_=agg1t[c][:])
            warm(3)
            ph1 = mmp.tile([P, D], F32, tag="ph")
            mlp(ph1, [x[:] for x in xts1], w1_sb, b1_sb)
            epilogue(ph1, out_sb)
            nc.scalar.dma_start(out=out_d[:], in_=out_sb[:])

    nc.compile()
    return nc


def get_program():
    if "nc" not in _CACHE:
        _CACHE["nc"] = _build_program()
    return _CACHE["nc"]


def prepare_in_maps(features, W0, b0, W1, b1, nodes2, neigh2, neigh1):
    """Host-side sharding + expanded transposed bf16/fp8 feature stream."""
    import ml_dtypes

    BF16 = ml_dtypes.bfloat16
    FP8 = ml_dtypes.float8_e4m3

    features = np.ascontiguousarray(features, dtype=np.float32)
    featsb = features.astype(BF16)
    feats8 = features.astype(FP8)

    # first const tile [128, 128]: the fp8 DoubleRow identity (tiny, so its
    # DMA semaphore fires before the first stream group lands)
    id2 = np.zeros((P, 2 * P), dtype=FP8)  # [p, (j, m)] DoubleRow identity
    id2[np.arange(P), np.arange(P)] = 1.0
    id2[np.arange(P), P + np.arange(P)] = 1.0
    cst0 = np.ascontiguousarray(id2.view(np.uint8).view(BF16))

    # critical consts [128, 1408]: w0 chunks, ones, b0
    csta = np.zeros((P, NCA), dtype=BF16)
    w0 = np.ascontiguousarray(W0.T, dtype=np.float32).copy()
    w0[D:, :] /= S0  # fold the layer-0 neighbor mean into the weights
    csta[:, 0:1024] = (
        w0.reshape(4, P, D).transpose(1, 0, 2).reshape(P, 1024).astype(BF16)
    )
    csta[0, 1024:1152] = 1.0  # ones row for the rank-1 bias matmul
    csta[0, 1152:1408] = b0.astype(BF16)

    # deferred consts [128, 2688]: w1 chunks, a1 selection, identity, b1
    cstb = np.zeros((P, NCB), dtype=BF16)
    w1 = np.ascontiguousarray(W1.T, dtype=np.float32).copy()
    w1[D:, :] /= S1
    cstb[:, 0:1024] = (
        w1.reshape(4, P, D).transpose(1, 0, 2).reshape(P, 1024).astype(BF16)
    )
    # layer-1 aggregation matrices: token 128*g + p (g>=1) is neighbor
    # j = 128*(g-1) + p of target j // 10
    a1 = np.zeros((S1, P, P), dtype=np.float32)
    j = np.arange(P * S1)
    a1[j // P, j % P, j // S1] = 1.0
    cstb[:, 1024:2304] = a1.transpose(1, 0, 2).reshape(P, S1 * P).astype(BF16)
    cstb[:, 2304:2432] = np.eye(P, dtype=np.float32).astype(BF16)
    cstb[0, 2432:2688] = b1.astype(BF16)

    in_maps = []
    bc = B // N_CORES  # 128 targets per core
    for c in range(N_CORES):
        nodes2_c = nodes2[c * bc:(c + 1) * bc]
        neigh2_c = neigh2[c * bc:(c + 1) * bc, :]
        nodes1_c = np.concatenate([nodes2_c, neigh2_c.reshape(-1)])
        neigh1_c = np.concatenate(
            [
                neigh1[c * bc:(c + 1) * bc, :],
                neigh1[B + c * bc * S1:B + (c + 1) * bc * S1, :],
            ],
            axis=0,
        )
        # self stream: [g, t, (c, f)] -> [g, f, c, t] bf16
        selfT = (
            featsb[nodes1_c]
            .reshape(NG, P, 2, P)
            .transpose(0, 3, 2, 1)
        )
        self_u8 = np.ascontiguousarray(selfT).view(np.uint8).reshape(NG, P, -1)
        # neighbor stream: [g, t, s, (c, f)] -> [g, f, s, c, t] fp8
        nbT = (
            feats8[neigh1_c.reshape(-1)]
            .reshape(NG, P, S0, 2, P)
            .transpose(0, 4, 2, 3, 1)
        )
        nb_u8 = np.ascontiguousarray(nbT).view(np.uint8).reshape(NG, P, -1)
        st = np.concatenate([self_u8, nb_u8], axis=2).view(np.int8)
        in_maps.append({"st": st, "cst0": cst0, "csta": csta, "cstb": cstb})
    return in_maps


def kernel(features, W0, b0, W1, b1, nodes2, neigh2, neigh1, _trace=False):
    from concourse.bass_utils import run_bass_kernel_spmd

    nc = get_program()
    in_maps = prepare_in_maps(features, W0, b0, W1, b1, nodes2, neigh2, neigh1)
    kwargs = {}
    if _trace:
        import tempfile

        import ntff_shim  # noqa: F401  (registers the axon NTFF hook)

        kwargs = {"trace": True, "tmpdir": tempfile.mkdtemp(prefix="ntff_")}
    res = run_bass_kernel_spmd(nc, in_maps, list(range(N_CORES)), **kwargs)
    out = np.concatenate([res.results[c]["out"] for c in range(N_CORES)], axis=0)
    if _trace:
        _CACHE["last_result"] = res
    return out
